# revision 21
# baseline (speedup 1.0000x reference)
"""Causal self-attention, head-tensor-parallel across 8 TRN2 NeuronCores.

Problem: x[2,2048,1024] -> qkv = x@W_attn+b_attn -> 16-head causal attention
(head dim 64) -> y@W_proj+b_proj.

Sharding: heads are tensor-parallel. Core c owns heads 2c and 2c+1:
  - W_attn column slices for its q/k/v features (384 cols), W_proj row slice
    (128 rows). Every core reads all of x (transposed+bf16 on host).
  - Each core emits a full [4096,1024] fp16 partial of the output projection;
    the host sums the 8 partials and adds b_proj.

On-core dataflow (all matmuls bf16 in / fp32 PSUM accum):
  1. qkv^T[384,4096] = W_slice^T @ x^T (features on partitions).  q,k of
     batch 0 run first so the exp stream starts ~24us in; v / batch-1 /
     V-transposes fill PE gaps under the exp-paced attention phase.
  2. V = PE-transpose of v^T, augmented with a ones column (row sums of
     P fall out of the AV matmul as column 64 -> softmax denominator).
  3. S^T[k,q] = k^T.T @ q^T per head, causally block-skipped; the two heads
     run row-group-packed (contraction K=64 at partitions 0-63 / 64-127,
     adjacent fills -> concurrent row groups).  exp via ScalarE (scale 1/8).
     P^T is packed Q-MAJOR: per 512-token q-chunk qc, the (kt<=4qc+3) [k,q]
     blocks lie contiguously.  A q-chunk's region is written by exp(b,qc)
     and read ONLY by AV(b,qc), so exp(b1,qc) needs just AV(b0,qc) done:
     the ScalarE stream runs gap-free across batches while AV/normalize/
     projection trail one q-chunk behind.  Diagonal blocks get a triu mask
     multiply after exp.  Head-alternating [128,1024] PSUM windows (2-buf
     pool, 4 banks) keep exp at full rate.
  4. y_aug^T[65,512] per (head, q-chunk): V_aug stationary, P^T moving,
     one matmul per kt (contiguous block), accumulated in PSUM.  Normalized
     IN TRANSPOSED ORIENTATION (no PE transposes): denom row 64 ->
     partition 0 (DVE copy), reciprocal_approx_fast (DVE),
     partition_broadcast (GpSimd), DVE multiply -> yT_sb bf16.
  5. out[tok,1024] = yT.T @ W_proj per 128-token tile, right behind each
     q-chunk's normalize; evict fp16 (DVE/ScalarE alternating), DMA.

PSUM is statically planned: s_acc 2x[128,1024]f32 (4 banks) + one shared
4-slot work ring (qkv evictions, V transposes, y_aug, proj) = 8 banks,
so no pool-handoff release gates AV behind tail QKV work.
"""

import numpy as np
import ml_dtypes

import concourse.bacc as bacc
import concourse.bass as bass
import concourse.mybir as mybir
import concourse.tile as tile
from concourse.bass_utils import run_bass_kernel_spmd
from concourse.masks import make_identity

BF16 = mybir.dt.bfloat16
FP16 = mybir.dt.float16
FP32 = mybir.dt.float32

B, T, C, H = 2, 2048, 1024, 16
D = C // H            # 64
N_CORES = 8
HPC = H // N_CORES    # heads per core = 2
TOK = B * T           # 4096
P = 128               # partitions / tile edge
KT = T // P           # 16 k/q tiles per batch element
NQ = 1024             # exp window width (2 PSUM banks)
XC = 512              # x^T token chunk for streaming
NCHB = T // XC        # chunks per batch = 4
CW = (C // P) * XC    # flat per-partition chunk width = 4096
QQ = 512              # AV / q-chunk width
NQC = T // QQ         # q-chunks per batch = 4

bf16 = ml_dtypes.bfloat16


def _layout():
    """Q-major packed P^T layout: per q-chunk qc, blocks kt=0..4qc+3."""
    boff, qcr = {}, []
    off = 0
    for qc in range(NQC):
        start = off
        for kt in range(4 * qc + 4):
            w = min(QQ, QQ * (qc + 1) - P * kt)
            boff[(qc, kt)] = off
            off += w
        qcr.append((start, off - start))
    return boff, qcr, off


BOFF, QCR, PT_COLS = _layout()  # PT_COLS = 17408


def build_nc():
    nc = bacc.Bacc("TRN2", target_bir_lowering=False, debug=False)

    # All DRAM inputs host-prepacked so every DMA is a flat 2D pattern with
    # multi-KB contiguous runs per partition.
    xT = nc.dram_tensor("xT", [TOK // XC, P, CW], BF16, kind="ExternalInput").ap()
    w_qkv = nc.dram_tensor(
        "w_qkv", [P, (C // P) * 3 * P], BF16, kind="ExternalInput"
    ).ap()
    b_qkv = nc.dram_tensor("b_qkv", [P, 3], FP32, kind="ExternalInput").ap()
    w_p = nc.dram_tensor("w_p", [P, C], BF16, kind="ExternalInput").ap()
    out_p = nc.dram_tensor("out_p", [TOK, C], FP16, kind="ExternalOutput").ap()

    with tile.TileContext(nc) as tc:
        _emit(nc, tc, xT, w_qkv, b_qkv, w_p, out_p)
    nc.compile()
    return nc


def _emit(nc, tc, xT, w_qkv, b_qkv, w_p, out_p):
    from contextlib import ExitStack

    ctx = ExitStack()
    with ctx:
        consts = ctx.enter_context(tc.tile_pool(name="consts", bufs=1))
        persist = ctx.enter_context(tc.tile_pool(name="persist", bufs=1))

        # ---- constants.  Trigger order matters: each DMA trigger costs
        # ~600ns on the Sync engine, so w + the x chunks go first (they gate
        # the first matmuls); bias/identity/w_p follow. ----
        w_qkv_sb = consts.tile([P, (C // P) * 3 * P], BF16)
        nc.sync.dma_start(out=w_qkv_sb, in_=w_qkv)
        bias_sb = consts.tile([P, 3], FP32)
        ident = consts.tile([P, P], BF16)
        w_p_sb = consts.tile([P, C], BF16)  # DMA emitted after S(b0)

        # ---- persistent activations ----
        qT = persist.tile([P, TOK], BF16)   # rows: head A dims 0-63, head B 64-127
        kTt = persist.tile([P, TOK], BF16)
        vT = persist.tile([P, TOK], BF16)
        qkvT = [qT, kTt, vT]
        v_aug = [
            [persist.tile([P, KT, D + 1], BF16, name=f"v_aug_{b}_{h}") for h in range(HPC)]
            for b in range(B)
        ]
        yT_sb = persist.tile([P, TOK], BF16)    # normalized y^T, feat on partitions
        # q-major packed P^T store, one per head (reused across b; region qc
        # is written by exp(b,qc) and read only by AV(b,qc))
        pt_sb = [persist.tile([P, PT_COLS], BF16, name=f"pt_{h}") for h in range(HPC)]

        # ---- pools (PSUM static: s 4 banks + work 4 banks) ----
        osb = ctx.enter_context(tc.tile_pool(name="o_sb", bufs=4))
        rcp = ctx.enter_context(tc.tile_pool(name="rc_rows", bufs=4))
        bcp = ctx.enter_context(tc.tile_pool(name="bcast", bufs=4))
        xp = tc.alloc_tile_pool(name="xT_pool", bufs=5)
        # PSUM static plan: s(4) + qv(2) + tail(2) = 8 banks.  qv and tail
        # are SEPARATE rings so AV/proj slot reuse never chains behind
        # tail-QKV / V-transpose work from the other pipeline.
        sps = tc.alloc_tile_pool(name="s_ps", bufs=2, space="PSUM")
        qvp = tc.alloc_tile_pool(name="qv_ps", bufs=2, space="PSUM")
        tlp = tc.alloc_tile_pool(name="tail_ps", bufs=2, space="PSUM")

        x_chunks = {}

        def fetch_x(nch):
            if nch not in x_chunks:
                x_sb = xp.tile([P, CW], BF16, name="x_sb")
                nc.sync.dma_start(out=x_sb, in_=xT[nch])
                x_chunks[nch] = x_sb

        def emit_qkv(nch, mis):
            fetch_x(nch)
            x_sb = x_chunks[nch]
            for mi in mis:
                ps = qvp.tile([P, XC], FP32, name="work")
                for kt in range(C // P):
                    nc.tensor.matmul(
                        ps,
                        w_qkv_sb[:, kt * 3 * P + mi * P : kt * 3 * P + (mi + 1) * P],
                        x_sb[:, kt * XC : (kt + 1) * XC],
                        start=(kt == 0),
                        stop=(kt == C // P - 1),
                    )
                nc.vector.tensor_scalar_add(
                    out=qkvT[mi][:, nch * XC : (nch + 1) * XC],
                    in0=ps,
                    scalar1=bias_sb[:, mi : mi + 1],
                )

        def emit_v(b):
            for h in range(HPC):
                nc.vector.memset(v_aug[b][h][:, :, D : D + 1], 1.0)
            for kt in range(KT):
                tok0 = b * T + kt * P
                ps_t = qvp.tile([P, P], BF16, name="work")
                nc.tensor.transpose(ps_t, vT[:, tok0 : tok0 + P], ident)
                for h in range(HPC):
                    nc.vector.tensor_copy(
                        out=v_aug[b][h][:, kt, 0:D],
                        in_=ps_t[:, h * D : (h + 1) * D],
                    )

        def emit_s(b, qc):
            """S^T fill + exp for q-chunk qc: [128,1024] windows over the
            qc-region; both heads' fills adjacent (concurrent row groups),
            exps head-alternating through the 2-buf pool."""
            start, width = QCR[qc]
            masked = set()
            for c0 in range(start, start + width, NQ):
                c1 = min(c0 + NQ, start + width)
                ps_pair = []
                for h in range(HPC):
                    ps_s = sps.tile([P, NQ], FP32, name="s_acc")
                    rows = slice(h * D, (h + 1) * D)
                    for kt in range(4 * qc + 4):
                        bo = BOFF[(qc, kt)]
                        w = min(QQ, QQ * (qc + 1) - P * kt)
                        a, bnd = max(c0, bo), min(c1, bo + w)
                        if a >= bnd:
                            continue
                        q_base = max(QQ * qc, P * kt)
                        c = a
                        while c < bnd:
                            nxt = min(bnd, c0 + ((c - c0) // 512 + 1) * 512)
                            q0 = q_base + (c - bo)
                            nc.tensor.matmul(
                                ps_s[:, c - c0 : nxt - c0],
                                kTt[rows, b * T + kt * P : b * T + kt * P + P],
                                qT[rows, b * T + q0 : b * T + q0 + nxt - c],
                                start=True,
                                stop=True,
                                tile_position=(h * D, 0),
                            )
                            c = nxt
                    ps_pair.append(ps_s)
                for h in range(HPC):
                    nc.scalar.activation(
                        out=pt_sb[h][:, c0:c1],
                        in_=ps_pair[h][:, 0 : c1 - c0],
                        func=mybir.ActivationFunctionType.Exp,
                        scale=1.0 / np.sqrt(D),
                    )
                # triu masks for diagonal blocks fully covered so far
                for kt in range(4 * qc, 4 * qc + 4):
                    bo = BOFF[(qc, kt)]
                    if kt in masked or bo + P > c1:
                        continue
                    masked.add(kt)
                    for h in range(HPC):
                        nc.gpsimd.affine_select(
                            out=pt_sb[h][:, bo : bo + P],
                            in_=pt_sb[h][:, bo : bo + P],
                            pattern=[[1, P]],
                            compare_op=mybir.AluOpType.is_ge,
                            fill=0.0,
                            base=0,
                            channel_multiplier=-1,
                        )

        def emit_proj(gq):
            o_sb = osb.tile([P, C], FP16, name="o_stage")
            for fj in range(C // 512):
                ps_o = tlp.tile([P, 512], FP32, name="tail")
                nc.tensor.matmul(
                    ps_o,
                    yT_sb[:, gq * P : (gq + 1) * P],
                    w_p_sb[:, fj * 512 : (fj + 1) * 512],
                    start=True,
                    stop=True,
                )
                # DVE-only evictions: ScalarE's FIFO stays pure-exp so the
                # cross-batch activation stream never stalls behind copies
                nc.vector.tensor_copy(out=o_sb[:, fj * 512 : (fj + 1) * 512], in_=ps_o)
            nc.sync.dma_start(out=out_p[gq * P : (gq + 1) * P, :], in_=o_sb)

        def emit_av(b, qc):
            """AV + transposed-orientation normalize + projection for qc."""
            q0, q1 = qc * QQ, (qc + 1) * QQ
            for h in range(HPC):
                ps_ya = tlp.tile([D + 1, QQ], FP32, name="tail")
                kmax = 4 * qc + 3
                for kt in range(kmax + 1):
                    bo = BOFF[(qc, kt)]
                    w = min(QQ, QQ * (qc + 1) - P * kt)
                    sub0 = max(q0, kt * P)
                    nc.tensor.matmul(
                        ps_ya[:, sub0 - q0 : QQ],
                        v_aug[b][h][:, kt, :],
                        pt_sb[h][:, bo : bo + w],
                        start=(kt == 0),
                        stop=(kt == kmax),
                    )
                dn = rcp.tile([1, QQ], FP32, name="dn_row")
                nc.vector.tensor_copy(out=dn, in_=ps_ya[D : D + 1, :])
                rc = rcp.tile([1, QQ], FP32, name="rc_row")
                nc.vector.reciprocal_approx_fast(rc, dn)
                bc = bcp.tile([D, QQ], FP32, name="bc")
                nc.gpsimd.partition_broadcast(bc, rc, channels=D)
                nc.vector.tensor_tensor(
                    out=yT_sb[h * D : (h + 1) * D, b * T + q0 : b * T + q1],
                    in0=ps_ya[0:D, :],
                    in1=bc,
                    op=mybir.AluOpType.mult,
                )
            for j in range(QQ // P):
                emit_proj(b * KT + qc * (QQ // P) + j)

        # ---- pipeline (emission order = scheduling priority) ----
        for nch in range(NCHB):
            fetch_x(nch)                # all b0 x-chunk DMA triggers first
        nc.sync.dma_start(out=bias_sb, in_=b_qkv)
        make_identity(nc, ident)
        for nch in range(NCHB):
            emit_qkv(nch, (0, 1))       # q,k of batch 0 first
        for qc in range(NQC):
            emit_s(0, qc)
        nc.sync.dma_start(out=w_p_sb, in_=w_p)
        for nch in range(NCHB):
            emit_qkv(nch, (2,))         # v(b0) + below: PE gap fill
        emit_v(0)
        for nch in range(NCHB, 2 * NCHB):
            emit_qkv(nch, (0, 1, 2))
        emit_v(1)
        for qc in range(NQC):
            emit_av(0, qc)
        for qc in range(NQC):
            emit_s(1, qc)
        for qc in range(NQC):
            emit_av(1, qc)
        tlp.release()
        qvp.release()
        sps.release()
        xp.release()


def shard_inputs(x, W_attn, b_attn, W_proj, b_proj):
    x = np.asarray(x, np.float32)
    W_attn = np.asarray(W_attn, np.float32)
    b_attn = np.asarray(b_attn, np.float32)
    W_proj = np.asarray(W_proj, np.float32)

    xT = np.ascontiguousarray(
        x.reshape(TOK // XC, XC, C // P, P).transpose(0, 3, 2, 1).reshape(
            TOK // XC, P, CW
        )
    ).astype(bf16)
    in_maps = []
    for c in range(N_CORES):
        w_cols = np.stack(
            [W_attn[:, m * C + P * c : m * C + P * (c + 1)] for m in range(3)],
            axis=1,
        )  # [C, 3, P]
        w_slice = np.ascontiguousarray(
            w_cols.reshape(C // P, P, 3 * P).transpose(1, 0, 2).reshape(P, -1)
        ).astype(bf16)
        b_slice = np.ascontiguousarray(
            np.stack(
                [b_attn[m * C + P * c : m * C + P * (c + 1)] for m in range(3)],
                axis=1,
            )
        ).astype(np.float32)  # [P, 3]
        wp_slice = np.ascontiguousarray(W_proj[P * c : P * (c + 1), :]).astype(bf16)
        in_maps.append(
            {"xT": xT, "w_qkv": w_slice, "b_qkv": b_slice, "w_p": wp_slice}
        )
    return in_maps


def kernel(x, W_attn, b_attn, W_proj, b_proj, _trace=False):
    in_maps = shard_inputs(x, W_attn, b_attn, W_proj, b_proj)
    nc = build_nc()
    res = run_bass_kernel_spmd(nc, in_maps, list(range(N_CORES)), trace=_trace)
    acc = np.zeros((TOK, C), np.float64)
    for r in res.results:
        acc += r["out_p"].astype(np.float64)
    out = acc.astype(np.float32) + np.asarray(b_proj, np.float32)[None, :]
    if _trace:
        kernel.last_results = res
    return out.reshape(B, T, C)
